# revision 4
# baseline (speedup 1.0000x reference)
"""Trainium2 Bass kernel for nn_CDAN_Dis (CDAN discriminator head), v2.

Math per sample m (see reference):
  a    = einsum('cf,bft->bct', w2d, feature)            # [C,T]
  d    = einsum('bct,bcpt->bpt', a, mask) + b2d         # [P,T]
  d    = leaky(GLN_scalar(d))                           # global LN over (P,T)
  x1   = leaky(GLN_vec(conv1d(d,  w1,b1, s2,p1)))       # [256,1000]
  x2   = leaky(GLN_vec(conv1d(x1, w2,b2, s2,p1)))       # [256,500]
  out  = conv1d(x2, w3, b3, s1, p0)                     # [1,500]

Data-parallel over batch M=4 across 4 NeuronCores (one sample per core).

v2 design notes:
 - All streamed tensors (feature, mask, weights) are cast to fp16 on the
   host: halves HBM traffic (memory-regime problem) and runs matmuls at
   1 cyc/row without f32r quirks.  Stats/accumulators stay f32.
 - The build is software-pipelined: iteration i's stage-1 (DMA + DVE
   multiplies, the engines the back half barely uses) is emitted
   interleaved into iteration i-1's conv/norm phases, so the in-order
   per-engine queues allow cross-iteration overlap under repeat-K.
 - GLN stats: per-partition partial sums/sumsqs ride accum_out on the
   producing instructions; the cross-partition + group reduction is a set
   of accumulated f32 ones[128,128] matmuls into a [128,2] psum, and the
   mean/var/sstd chain runs as consecutive ACT ops on all 128 partitions
   (variance subtraction rides the per-partition bias operand), so no
   [1,*] stage and no re-broadcast.
 - norm1 (GLN1 normalize+leaky) is split into mod-4 column phases whose
   accum_outs are the phase sums of x; conv1's output SUM then collapses
   to a dot with host-precomputed column-summed w1 (no reduce pass):
     sum_t y[p,t] = sum_{q,k} w1s[q,k]*S_k[q] + T1*sum(b1)
   conv2's output sums use direct per-bank DVE reduces instead.
 - Engine notes learned the hard way (the real compiler rejects what
   CoreSim happily simulates): gpsimd/Pool only runs plain tensor add/mul/
   copy (no accum_out, no tensor_scalar, no PSUM operands); DVE
   tensor_tensor_reduce crashes the device at runtime; DVE instructions
   may read at most one PSUM operand; tensor_scalar pow/divide fail ISA
   checks.  Hence sumsq passes use ACT Square+accum and the leaky fallback
   pair stays on DVE.
 - b2d is a uniform additive constant immediately followed by a global
   layernorm, so it cancels exactly and is ignored.
"""

import sys

sys.path.insert(0, "/opt/trn_rl_repo")

from contextlib import ExitStack

import numpy as np

import concourse.bass as bass
import concourse.mybir as mybir
import concourse.tile as tile
from concourse import bacc, bass_utils

F32 = mybir.dt.float32
F16 = mybir.dt.float16
AX = mybir.AxisListType
OP = mybir.AluOpType
AF = mybir.ActivationFunctionType

M, C, B, T = 4, 2, 128, 2000
TC = 500               # matmul free-dim chunk (PSUM bank limit)
NCHUNK = T // TC       # 4
T1 = 1000              # conv1 output length
T2 = 500               # conv2 output length
EPS = 1e-8

N1 = B * T             # GLN1 element count
N2 = 256 * T1
N3 = 256 * T2

USE_PRELU = True       # fused affine+leaky on ACT (Prelu alpha=0.1)
WARM_S1 = 1            # dummy matmuls per stage-1 chunk (PE p-state)
WARM_CHAIN = 1         # dummy matmuls per GLN chain
N_CORES = 4

# packed fp16 weights column offsets (CW: [128, CWW] f16)
CW_W2DR = 0            # w2d broadcast lhsT [128, 256]
CW_W1T = 256           # w1 transposed [128, 768]
CW_W2T = 1024          # w2 transposed [128, 1536]
CW_W3T = 2560          # w3 cols + pad for 128-col lhsT reads
CWW = 2692
# packed f32 per-partition constants (CWF: [128, CWF_W] f32)
CF_ONES = 0            # ones [128, 128] (stats-bcast lhsT)
CF_W1S = 128           # w1.sum(out_ch) [128, 3]
CF_W2S = 131           # w2.sum(out_ch) reshaped [128, 6]
CF_G1 = 137            # gamma1 per oh-half [128, 2]
CF_BB1 = 139
CF_G2 = 141
CF_BB2 = 143
CF_EPS = 145
CF_CB1 = 146           # T1*sum(b1)/128
CF_CB2 = 147           # T2*sum(b2)/128
CF_G2D = 148
CF_BE2D = 149
CF_B3 = 150
CWF_W = 152
# packed fp16 rows (RW: [1, RWW] f16)
RW_B1 = 0
RW_B2 = 256
RW_ONES = 512          # 500 ones
RW_B3R = 1024          # b3 broadcast [1,128] (conv3 bias matmul lhsT)
RWW = 1280


def _patch_act_tables():
    """Pin every ACT func we use to the one set that has them all."""
    if getattr(bacc, "_cdan_act_patch", False):
        return
    orig = bacc.get_activation_tables
    mine = {AF.Copy, AF.Identity, AF.Square, AF.Sqrt, AF.Prelu}

    def patched(arch):
        t = dict(orig(arch))
        for name in t:
            if name != "sqrt_and_others":
                t[name] = set(t[name]) - mine
        return t

    bacc.get_activation_tables = patched
    bacc._cdan_act_patch = True


def build_nc(repeat=1):
    _patch_act_tables()
    nc = bacc.Bacc("TRN2", target_bir_lowering=False, debug=False,
                   num_devices=N_CORES)

    feature_d = nc.dram_tensor("feature", [B, T], F16, kind="ExternalInput").ap()
    mask_d = nc.dram_tensor("mask", [C, B, T], F16, kind="ExternalInput").ap()
    cw_d = nc.dram_tensor("cw", [128, CWW], F16, kind="ExternalInput").ap()
    cwf_d = nc.dram_tensor("cwf", [128, CWF_W], F32, kind="ExternalInput").ap()
    rw_d = nc.dram_tensor("rw", [1, RWW], F16, kind="ExternalInput").ap()
    out_d = nc.dram_tensor("out", [1, T2], F32, kind="ExternalOutput").ap()

    with tile.TileContext(nc) as tc:
        with ExitStack() as ctx:
            pools = _make_pools(ctx, tc)
            # software pipeline: iteration i's stage-1 (DMA/DVE/Pool-heavy)
            # is emitted interleaved into iteration i-1's conv/norm phases
            # (ACT/PE-heavy), so per-engine program order permits overlap.
            prev = None
            for _ in range(repeat):
                st = _emit_A_dmas(pools, tc, feature_d, mask_d, cw_d,
                                  cwf_d, rw_d, out_d)
                if prev is None:
                    for j in range(NCHUNK):
                        _emit_A_chunk(pools, tc, st, j)
                else:
                    gen = _emit_B(pools, tc, prev)
                    next(gen)              # chain1 + norm1 of prev
                    for j in range(NCHUNK):
                        next(gen)          # next conv/chain phase of prev
                        _emit_A_chunk(pools, tc, st, j)
                    for _ in gen:          # conv3 + out of prev
                        pass
                prev = st
            for _ in _emit_B(pools, tc, prev):
                pass
    nc.compile()
    return nc


def _make_pools(ctx, tc):
    class P:
        pass
    p = P()
    p.const = ctx.enter_context(tc.tile_pool(name="const", bufs=2))
    p.inp = ctx.enter_context(tc.tile_pool(name="inp", bufs=2))
    p.tmpp = ctx.enter_context(tc.tile_pool(name="tmpp", bufs=4))
    p.sqp = ctx.enter_context(tc.tile_pool(name="sqp", bufs=2))
    p.bigp = ctx.enter_context(tc.tile_pool(name="bigp", bufs=2))
    p.smallp = ctx.enter_context(tc.tile_pool(name="smallp", bufs=2))
    p.psmm = ctx.enter_context(tc.tile_pool(name="psmm", bufs=4, space="PSUM"))
    p.psa = ctx.enter_context(tc.tile_pool(name="psa", bufs=2, space="PSUM"))
    p.psS = ctx.enter_context(tc.tile_pool(name="psS", bufs=1, space="PSUM"))
    p.ps1 = ctx.enter_context(tc.tile_pool(name="ps1", bufs=1, space="PSUM"))
    return p


def _warm(nc, pools, lhsT, rhs, n):
    # keeps the PE p-state hot; shares the conv3 psum bank (write-only)
    for _ in range(n):
        wt = pools.ps1.tile([128, TC], F32, tag="mm1")
        nc.tensor.matmul(wt[:], lhsT, rhs, start=True, stop=True)


def _gln_chain(nc, pools, stab, onesq, epsc, inv_n, tag):
    """stab [128,8] = (sum parts 0:4 | sumsq parts 4:8) -> (rstd, negmean).

    The cross-partition + group reduction is 4 accumulated ones-matmuls
    into a [128,2] psum; the mean/var/sstd math then runs as 4 consecutive
    ACT ops (no cross-engine hops; the variance subtraction rides the
    per-partition bias operand).  All partitions hold the same values."""
    smallp, psS = pools.smallp, pools.psS
    w = stab[:].shape[-1] // 2
    bc2 = psS.tile([128, 2], F32, tag="bc")
    for k in range(w):
        nc.tensor.matmul(bc2[:], onesq, stab[:, k:2 * w:w],
                         start=(k == 0), stop=(k == w - 1))
    mE = smallp.tile([128, 2], F32, tag=f"mE{tag}")
    nc.scalar.activation(mE[:], bc2[:], AF.Copy, bias=0.0, scale=-inv_n)
    sqm = smallp.tile([128, 1], F32, tag=f"sqm{tag}")
    nc.scalar.activation(sqm[:], mE[:, 0:1], AF.Square)
    nvar = smallp.tile([128, 1], F32, tag=f"nvar{tag}")
    nc.scalar.activation(nvar[:], sqm[:], AF.Identity,
                         bias=mE[:, 1:2], scale=1.0)      # mean^2 - E2
    sstd = smallp.tile([128, 1], F32, tag=f"sstd{tag}")
    nc.scalar.activation(sstd[:], nvar[:], AF.Sqrt, bias=epsc, scale=-1.0)
    rstd = smallp.tile([128, 1], F32, tag=f"rstd{tag}")
    nc.vector.reciprocal(rstd[:], sstd[:])
    return rstd, mE[:, 0:1]


def _scale_bias(nc, pools, rstd, negmean, gam, bet, w, tag):
    """scale = gamma*rstd; bias = scale*(-mean) + beta   ([128, w])."""
    smallp = pools.smallp
    sc = pools.smallp.tile([128, w], F32, tag=f"sc{tag}")
    nc.vector.tensor_scalar_mul(sc[:], gam, rstd)
    bi = smallp.tile([128, w], F32, tag=f"bi{tag}")
    nc.vector.scalar_tensor_tensor(bi[:], sc[:], negmean, bet,
                                   OP.mult, OP.add)
    return sc, bi


def _norm_leaky(nc, pools, out_ap, in_ap, scale_ap, bias_ap, accum=None):
    """out = leaky(in*scale + bias), slope 0.1; optional sum accumulator."""
    if USE_PRELU:
        nc.scalar.activation(out_ap, in_ap, AF.Prelu,
                             bias=bias_ap, scale=scale_ap, alpha=0.1,
                             accum_out=accum)
    else:
        af = pools.tmpp.tile([128, out_ap.shape[-1]], F32, tag="nl")
        nc.scalar.activation(af[:], in_ap, AF.Identity,
                             bias=bias_ap, scale=scale_ap)
        nc.vector.scalar_tensor_tensor(out_ap, af[:], 0.1, af[:],
                                       OP.mult, OP.max, accum_out=accum)


def _norm_leaky_pool(nc, pools, out_ap, in_ap, scale_ap, bias_ap, accum=None,
                     n=TC):
    """Same as _norm_leaky but on the Pool engine (SBUF operands only):
    z = in*scale + bias (tensor_scalar); out = max(z, 0.1*z) (stt)."""
    z = pools.tmpp.tile([128, n], F16, tag="plz")
    nc.gpsimd.tensor_scalar(z[:, 0:n], in_ap, scale_ap, bias_ap,
                            OP.mult, OP.add)
    nc.gpsimd.scalar_tensor_tensor(out_ap, z[:, 0:n], 0.1, z[:, 0:n],
                                   OP.mult, OP.max, accum_out=accum)


SECTION_LOG = None


def _mark(nc, label):
    if SECTION_LOG is not None:
        SECTION_LOG.append((label, nc.next_id()))


class _St:
    """Per-iteration emission state."""


def _emit_A_dmas(pools, tc, feature_d, mask_d, cw_d, cwf_d, rw_d, out_d):
    """Emit iteration i's input DMA triggers; allocate its big tiles."""
    nc = tc.nc
    const, inp = pools.const, pools.inp
    st = _St()
    _mark(nc, "A_dmas")
    st.out_d = out_d
    cw = const.tile([128, CWW], F16, tag="cw")
    nc.sync.dma_start(cw[:, 0:256], cw_d[:, 0:256])          # w2dr early
    cwf = const.tile([128, CWF_W], F32, tag="cwf")
    nc.sync.dma_start(cwf[:], cwf_d[:])
    feat = inp.tile([128, T], F16, tag="feat")
    m0 = inp.tile([128, T], F16, tag="m0")
    m1 = inp.tile([128, T], F16, tag="m1")
    nc.sync.dma_start(feat[:, 0:1000], feature_d[:, 0:1000])
    nc.sync.dma_start(m0[:, 0:1000], mask_d[0, :, 0:1000])
    nc.sync.dma_start(m1[:, 0:1000], mask_d[1, :, 0:1000])
    nc.sync.dma_start(feat[:, 1000:2000], feature_d[:, 1000:2000])
    nc.sync.dma_start(m0[:, 1000:2000], mask_d[0, :, 1000:2000])
    nc.sync.dma_start(m1[:, 1000:2000], mask_d[1, :, 1000:2000])
    rwt = const.tile([1, RWW], F16, tag="rw")
    nc.sync.dma_start(rwt[:], rw_d[:])
    nc.sync.dma_start(cw[:, 256:CWW], cw_d[:, 256:CWW])

    st.cw, st.cwf, st.rwt = cw, cwf, rwt
    st.feat, st.m0, st.m1 = feat, m0, m1
    st.w2dr = cw[:, CW_W2DR:CW_W2DR + 256]
    st.w1t = cw[:, CW_W1T:CW_W1T + 768]
    st.w2t = cw[:, CW_W2T:CW_W2T + 1536]
    st.onesq = cwf[:, CF_ONES:CF_ONES + 128]
    st.w1s = cwf[:, CF_W1S:CF_W1S + 3]
    st.w2s = cwf[:, CF_W2S:CF_W2S + 6]
    st.g1f = cwf[:, CF_G1:CF_G1 + 2]
    st.bb1f = cwf[:, CF_BB1:CF_BB1 + 2]
    st.g2f = cwf[:, CF_G2:CF_G2 + 2]
    st.bb2f = cwf[:, CF_BB2:CF_BB2 + 2]
    st.epsc = cwf[:, CF_EPS:CF_EPS + 1]
    st.cb1c = cwf[:, CF_CB1:CF_CB1 + 1]
    st.cb2c = cwf[:, CF_CB2:CF_CB2 + 1]
    st.g2dc = cwf[:, CF_G2D:CF_G2D + 1]
    st.be2dc = cwf[:, CF_BE2D:CF_BE2D + 1]
    st.b3c = cwf[:, CF_B3:CF_B3 + 1]
    st.b1r = rwt[0:1, RW_B1:RW_B1 + 256]
    st.b2r = rwt[0:1, RW_B2:RW_B2 + 256]
    st.ones500 = rwt[0:1, RW_ONES:RW_ONES + T2]

    st.d = pools.bigp.tile([128, T], F16, tag="d")
    st.stab1 = pools.smallp.tile([128, 8], F32, tag="stab1")
    return st


def _emit_A_chunk(pools, tc, st, j):
    """Stage-1 chunk j: a-bcast matmuls, DVE mask muls, Pool add+sq stats."""
    nc = tc.nc
    _mark(nc, f"A_chunk{j}")
    sl = slice(j * TC, (j + 1) * TC)
    a0 = pools.psa.tile([128, TC], F32, tag="aps")
    nc.tensor.matmul(a0[:], st.w2dr[:, 0:128], st.feat[:, sl],
                     start=True, stop=True)
    a1 = pools.psa.tile([128, TC], F32, tag="aps")
    nc.tensor.matmul(a1[:], st.w2dr[:, 128:256], st.feat[:, sl],
                     start=True, stop=True)
    t0 = pools.tmpp.tile([128, TC], F16, tag="t0")
    nc.vector.tensor_mul(t0[:], st.m0[:, sl], a0[:])
    t1 = pools.tmpp.tile([128, TC], F16, tag="t1")
    nc.vector.tensor_mul(t1[:], st.m1[:, sl], a1[:])
    nc.vector.scalar_tensor_tensor(st.d[:, sl], t0[:], 0.0, t1[:],
                                   OP.add, OP.add,
                                   accum_out=st.stab1[:, j:j + 1])
    sq = pools.sqp.tile([128, TC], F16, tag="sq")
    if j % 2 == 0:
        nc.scalar.activation(sq[:], st.d[:, sl], AF.Square,
                             accum_out=st.stab1[:, 4 + j:5 + j])
    else:
        nc.vector.scalar_tensor_tensor(sq[:], st.d[:, sl], 0.0, st.d[:, sl],
                                       OP.add, OP.mult,
                                       accum_out=st.stab1[:, 4 + j:5 + j])
    _warm(nc, pools, st.w2dr[:, 0:128], st.feat[:, sl], WARM_S1)


def _emit_B(pools, tc, st):
    """Generator: GLN1 through conv3/out for iteration st, yielding at the
    four interleave points where the next iteration's stage-1 chunks slot
    in."""
    nc = tc.nc
    sqp, bigp, smallp = pools.sqp, pools.bigp, pools.smallp
    psmm, ps1 = pools.psmm, pools.ps1

    _mark(nc, "chain1")
    rstd1, nm1 = _gln_chain(nc, pools, st.stab1, st.onesq, st.epsc,
                            1.0 / N1, "1")
    sc1, bi1 = _scale_bias(nc, pools, rstd1, nm1, st.g2dc, st.be2dc, 1, "1")
    _warm(nc, pools, st.w2dr[:, 0:128], st.feat[:, 0:TC], WARM_CHAIN)

    # mod-4 phase split of the GLN1 normalize: each instruction covers one
    # column phase of the whole width, so its accum IS that phase sum
    # (feeds the conv1 sum-trick).  Phases 0-2 on ACT, phase 3 on DVE.
    _mark(nc, "norm1")
    xpad = bigp.tile([128, T + 2], F16, tag="xpad")
    nc.vector.memset(xpad[:, 0:1], 0.0)
    accx = smallp.tile([128, 4], F32, tag="accx")
    for r in range(4):
        src = st.d[:, r:2000:4]
        dst = xpad[:, 1 + r:2001:4]
        acc = accx[:, r:r + 1]
        if r < 3:
            _norm_leaky(nc, pools, dst, src, sc1[:, 0:1], bi1[:, 0:1],
                        accum=acc)
        else:
            z = pools.tmpp.tile([128, TC], F16, tag="plz")
            nc.vector.tensor_scalar(z[:], src, sc1[:, 0:1], bi1[:, 0:1],
                                    OP.mult, OP.add)
            nc.vector.scalar_tensor_tensor(dst, z[:], 0.1, z[:],
                                           OP.mult, OP.max,
                                           accum_out=acc)
    yield

    # ---- conv1 (128->256, k3 s2 p1) + b1; sumsq stats on DVE/ACT ----
    # tcb-major so the tcb=0 pair starts as soon as xpad half 0 is ready
    _mark(nc, "conv1")
    stab2 = smallp.tile([128, 8], F32, tag="stab2")
    py1 = {}
    for tcb in range(2):
        for oh in range(2):
            p = psmm.tile([128, TC], F32, tag="mmout")
            py1[(oh, tcb)] = p
            for k in range(3):
                rhs = xpad[:, k + 2 * (tcb * T2): k + 2 * (tcb * T2) + 2 * T2 - 1:2]
                nc.tensor.matmul(p[:], st.w1t[:, k * 256 + oh * 128:
                                               k * 256 + oh * 128 + 128],
                                 rhs, start=(k == 0), stop=False)
            nc.tensor.matmul(p[:], st.b1r[:, oh * 128:oh * 128 + 128],
                             st.ones500[:], start=False, stop=True)
            idx = oh * 2 + tcb
            sq = sqp.tile([128, TC], F32, tag="sqa")
            nc.scalar.activation(sq[:], p[:], AF.Square,
                                 accum_out=stab2[:, 4 + idx:5 + idx])

    # conv1 sum-part: S_k from phase sums of x, dot with col-summed w1
    # (P_r = sum of both halves' phase accums; SxE = P0+P2, SxO = P1+P3;
    #  S = (SxO-xlast, SxE, SxO))
    _mark(nc, "V1")
    sx = smallp.tile([128, 2], F32, tag="sx")        # (SxE, SxO)
    nc.vector.tensor_add(sx[:], accx[:, 0:2], accx[:, 2:4])
    s1t = smallp.tile([128, 3], F32, tag="s1t")
    nc.vector.tensor_sub(s1t[:, 0:1], sx[:, 1:2], xpad[:, 2000:2001])
    nc.vector.tensor_copy(s1t[:, 1:3], sx[:, 0:2])
    nc.vector.tensor_mul(stab2[:, 0:3], st.w1s[:], s1t[:])
    nc.vector.tensor_copy(stab2[:, 3:4], st.cb1c)
    yield

    # ---- GLN2 chain + normalize ----
    _mark(nc, "chain2")
    rstd2, nm2 = _gln_chain(nc, pools, stab2, st.onesq, st.epsc,
                            1.0 / N2, "2")
    sc2, bi2 = _scale_bias(nc, pools, rstd2, nm2, st.g1f, st.bb1f, 2, "2")
    _warm(nc, pools, st.w2dr[:, 0:128], st.feat[:, 0:TC], WARM_CHAIN)

    # norm2: 3 banks on ACT, bank (0,1) on DVE (2-op); Pool sums each
    # cih's even/odd columns (conv2 sum-trick input) as soon as that
    # half's y1pad is written
    _mark(nc, "norm2")
    y1pad = []
    for oh in range(2):
        yp = bigp.tile([128, T1 + 2], F16, tag=f"y1pad{oh}")
        y1pad.append(yp)
        nc.vector.memset(yp[:, 0:1], 0.0)
        for tcb in range(2):
            dst = yp[:, 1 + tcb * T2: 1 + (tcb + 1) * T2]
            p = py1[(oh, tcb)]
            if oh == 0 and tcb == 1:
                z = pools.tmpp.tile([128, TC], F16, tag="nz2")
                nc.vector.tensor_scalar(z[:], p[:], sc2[:, oh:oh + 1],
                                        bi2[:, oh:oh + 1], OP.mult, OP.add)
                nc.vector.scalar_tensor_tensor(dst, z[:], 0.1, z[:],
                                               OP.mult, OP.max)
            else:
                _norm_leaky(nc, pools, dst, p[:],
                            sc2[:, oh:oh + 1], bi2[:, oh:oh + 1])
    yield

    # ---- conv2 (256->256, k3 s2 p1) + b2; sumsq on DVE/ACT ----
    _mark(nc, "conv2")
    stab3 = smallp.tile([128, 8], F32, tag="stab3")
    py2 = {}
    for oh in range(2):
        p = psmm.tile([128, TC], F32, tag="mmout")
        py2[oh] = p
        first = True
        for cih in range(2):
            for k in range(3):
                rhs = y1pad[cih][:, k: k + 2 * T2 - 1:2]
                nc.tensor.matmul(p[:], st.w2t[:, cih * 768 + k * 256 + oh * 128:
                                               cih * 768 + k * 256 + oh * 128 + 128],
                                 rhs, start=first, stop=False)
                first = False
        nc.tensor.matmul(p[:], st.b2r[:, oh * 128:oh * 128 + 128],
                         st.ones500[:], start=False, stop=True)
        # per-bank mean on DVE; sumsq as two ACT Square halves
        nc.vector.reduce_sum(stab3[:, oh:oh + 1], p[:], axis=AX.X)
        sqa = sqp.tile([128, 250], F32, tag="sqa2")
        nc.scalar.activation(sqa[:], p[:, 0:250], AF.Square,
                             accum_out=stab3[:, 4 + 2 * oh:5 + 2 * oh])
        sqb = sqp.tile([128, 250], F32, tag="sqb2")
        nc.scalar.activation(sqb[:], p[:, 250:500], AF.Square,
                             accum_out=stab3[:, 5 + 2 * oh:6 + 2 * oh])
    nc.vector.memset(stab3[:, 2:4], 0.0)
    yield

    # ---- GLN3 chain + normalize ----
    _mark(nc, "chain3")
    rstd3, nm3 = _gln_chain(nc, pools, stab3, st.onesq, st.epsc,
                            1.0 / N3, "3")
    sc3, bi3 = _scale_bias(nc, pools, rstd3, nm3, st.g2f, st.bb2f, 2, "3")
    _warm(nc, pools, st.w2dr[:, 0:128], st.feat[:, 0:TC], WARM_CHAIN)

    _mark(nc, "norm3")
    x3 = []
    for oh in range(2):
        xt = bigp.tile([128, T2], F16, tag=f"x3_{oh}")
        x3.append(xt)
        _norm_leaky(nc, pools, xt[:], py2[oh][:],
                    sc3[:, oh:oh + 1], bi3[:, oh:oh + 1])
    yield

    # ---- conv3 (256->1, k1) + b3 ----
    # lhsT is 128 consecutive CW columns whose col0 holds w3 for the half;
    # rows 1..127 of the psum accumulate garbage that we never read.  The
    # b3 bias rides a third matmul so the result DMAs straight from PSUM.
    _mark(nc, "conv3")
    p3 = ps1.tile([128, T2], F32, tag="mm1")
    nc.tensor.matmul(p3[:], st.cw[:, CW_W3T:CW_W3T + 128], x3[0][:],
                     start=True, stop=False)
    nc.tensor.matmul(p3[:], st.rwt[0:1, RW_B3R:RW_B3R + 128],
                     st.ones500[:], start=False, stop=False)
    nc.tensor.matmul(p3[:], st.cw[:, CW_W3T + 1:CW_W3T + 129], x3[1][:],
                     start=False, stop=True)
    out_s = smallp.tile([1, T2], F32, tag="out_s")
    nc.vector.tensor_scalar(out_s[:], p3[0:1, :], 1.0, 0.0, OP.mult, OP.add)
    nc.sync.dma_start(st.out_d[:], out_s[:])


def shard_inputs(inputs):
    """Full inputs -> per-core in_maps (host-side layout prep)."""
    f = {k: np.ascontiguousarray(np.asarray(v, dtype=np.float32))
         for k, v in inputs.items()}
    cw = np.zeros((128, CWW), np.float16)
    w2d = f["w2d"]
    cw[:, CW_W2DR:CW_W2DR + 128] = np.tile(w2d[0][:, None], (1, 128))
    cw[:, CW_W2DR + 128:CW_W2DR + 256] = np.tile(w2d[1][:, None], (1, 128))
    cw[:, CW_W1T:CW_W1T + 768] = f["w1"].transpose(1, 2, 0).reshape(128, 768)
    cw[:, CW_W2T:CW_W2T + 1536] = (
        f["w2"].transpose(1, 2, 0).reshape(2, 128, 3, 256)
        .transpose(1, 0, 2, 3).reshape(128, 1536))
    cw[:, CW_W3T:CW_W3T + 2] = f["w3"].reshape(2, 128).T

    cwf = np.zeros((128, CWF_W), np.float32)
    cwf[:, CF_ONES:CF_ONES + 128] = 1.0
    cwf[:, CF_W1S:CF_W1S + 3] = f["w1"].sum(axis=0)                # [128,3]
    # w2 [256out, 256in, 3] -> in-halves [2, 128, 3] col-sums -> [128, 6]
    w2s = f["w2"].sum(axis=0).reshape(2, 128, 3).transpose(1, 0, 2)
    cwf[:, CF_W2S:CF_W2S + 6] = w2s.reshape(128, 6)
    cwf[:, CF_G1:CF_G1 + 2] = f["g1"].reshape(2, 128).T
    cwf[:, CF_BB1:CF_BB1 + 2] = f["bb1"].reshape(2, 128).T
    cwf[:, CF_G2:CF_G2 + 2] = f["g2"].reshape(2, 128).T
    cwf[:, CF_BB2:CF_BB2 + 2] = f["bb2"].reshape(2, 128).T
    cwf[:, CF_EPS] = EPS
    cwf[:, CF_CB1] = T1 * float(f["b1"].sum()) / 128.0
    cwf[:, CF_CB2] = T2 * float(f["b2"].sum()) / 128.0
    cwf[:, CF_G2D] = float(f["g2d"].reshape(()))
    cwf[:, CF_BE2D] = float(f["be2d"].reshape(()))
    cwf[:, CF_B3] = float(f["b3"].reshape(()))

    rw = np.zeros((1, RWW), np.float16)
    rw[0, RW_B1:RW_B1 + 256] = f["b1"]
    rw[0, RW_B2:RW_B2 + 256] = f["b2"]
    rw[0, RW_ONES:RW_ONES + 500] = 1.0
    rw[0, RW_B3R:RW_B3R + 128] = float(f["b3"].reshape(()))

    in_maps = []
    for i in range(M):
        in_maps.append(dict(
            cw=cw, cwf=cwf, rw=rw,
            feature=np.ascontiguousarray(f["feature"][i].astype(np.float16)),
            mask=np.ascontiguousarray(f["mask"][i].astype(np.float16))))
    return in_maps


_NC = None


def kernel(**inputs):
    global _NC
    if _NC is None:
        _NC = build_nc()
    in_maps = shard_inputs(inputs)
    res = bass_utils.run_bass_kernel_spmd(_NC, in_maps,
                                          core_ids=list(range(N_CORES)))
    out = np.stack([res.results[i]["out"] for i in range(M)], axis=0)
    return out.astype(np.float32)
